# revision 56
# baseline (speedup 1.0000x reference)
"""Multi-head attention on 8 Trainium2 NeuronCores.

Sharding: data-parallel over batch (4) x tensor-parallel over head-groups (2).
Core c handles batch c//2, heads [8*(c%2), 8*(c%2)+8). Each core computes its
partial out-projection (over its 512 channels); host sums the pair per batch.

Device-side design (per core; everything post-input in bf16):
  - Q^T/K^T [512,2048] ch-major; V [2048, 8h*65] tok-major
    (65th col per head = ones -> softmax denominator rides attn@V)
  - per unit (head, 128-token chunk): scores^T via 16 K=64 matmuls into
    psum [128, 8*128] x2 groups; exp on ACT -> P^T bf16; mask-mul on DVE
    (bf16 2x mode)
  - attn@V token-major: psum[t, 65] slot accumulating 16 s'-chunk matmuls
    (N=65 instead of N=512: half the PE row cost of the d-major form)
  - normalize: DVE reciprocal of the denom col + tensor_scalar_mul (the
    per-token denominator is per-partition in this layout)
  - PE transpose (identity matmul) back to ch-major into a bf16 psum view,
    DVE copy -> onrm; out-proj Wo^T x onrm -> psum -> DVE copy -> DMA
  - schedule: scores issue one unit ahead of attnV/steal processing (the
    exp ping-pong never waits); prologue = K/Q och0 + V-proj inputs with
    critical-path-ordered DMA; V-proj, remaining K/Q och slabs, and the
    out-projection are stolen into the attention loop's PE slack; attnV
    is deferred past the full V-proj issue (program-order hazard)
  PSUM: s0,s1 [128,1024] + av [128,1024] + pj0,pj1 [128,512] = 8 banks.
  Inputs are host-pre-tiled (x by r-block, W by och-slab, mask by token
  chunk) so every DMA lands in its consumption layout.
"""
import sys

sys.path.insert(0, "/opt/trn_rl_repo")

import numpy as np
import ml_dtypes

import concourse.bass as bass
import concourse.mybir as mybir
import concourse.tile as tile
from concourse import bacc
from concourse.bass_utils import run_bass_kernel_spmd

D_MODEL = 1024
NUM_HEADS = 16
DK = 64
B, S = 4, 2048
NCORES = 8
OG = 512            # channels per head-group
HPG = 8             # heads per group
IC = D_MODEL // 128  # 8 contraction chunks
NTC = S // 128       # 16 token chunks
F32 = mybir.dt.float32
F32R = mybir.dt.float32r
BF16 = mybir.dt.bfloat16


def round_f32r(x: np.ndarray) -> np.ndarray:
    """Round fp32 to fp32r (11-bit mantissa, round-to-nearest-even)."""
    b = np.ascontiguousarray(x, dtype=np.float32).view(np.uint32).astype(np.uint64)
    lsb = (b >> 12) & 1
    b = (b + 0x7FF + lsb) & 0xFFFFF000
    return (b & 0xFFFFFFFF).astype(np.uint32).view(np.float32)


def build_module():
    nc = bacc.Bacc("TRN2", target_bir_lowering=False, debug=False,
                   num_devices=NCORES)
    # x^T packed host-side by r-block: XQ3/XK3 [r, p, i*512+t']
    # = x^T[i*128+p, r*512+t'] -- one transfer per proj r-round
    XQ3 = nc.dram_tensor("XQ3", [4, 128, IC * 512], BF16,
                         kind="ExternalInput").ap()
    XK3 = nc.dram_tensor("XK3", [4, 128, IC * 512], BF16,
                         kind="ExternalInput").ap()
    # XV4 [g, p, i*256+c] = x^T[i*128+p, g*256+c]  (g = tch-pair group)
    XV4 = nc.dram_tensor("XV4", [8, 128, IC * 256], BF16,
                         kind="ExternalInput").ap()
    # WQ/WK pre-tiled by och slab: [4, 128, IC*128]; WV whole [128, IC*OG]
    WQT = nc.dram_tensor("WQT", [4, 128, IC * 128], BF16,
                         kind="ExternalInput").ap()
    WKT = nc.dram_tensor("WKT", [4, 128, IC * 128], BF16,
                         kind="ExternalInput").ap()
    WVT = nc.dram_tensor("WVT", [128, IC * OG], BF16, kind="ExternalInput").ap()
    WOT = nc.dram_tensor("WOT", [OG, D_MODEL], BF16, kind="ExternalInput").ap()
    # mask tiled [tc, p, sc, t]: M4[tc, p, sc, t] = mask[tc*128+t, sc*128+p]
    M4 = nc.dram_tensor("M4", [NTC, 128, NTC, 128], BF16,
                        kind="ExternalInput").ap()
    BQK = nc.dram_tensor("BQK", [128, 8], F32, kind="ExternalInput").ap()
    # BVON [1, 640] f32r: cols 0-511 = bv, 512-639 = ones
    BVON = nc.dram_tensor("BVON", [1, 640], F32R, kind="ExternalInput").ap()
    IDENT = nc.dram_tensor("IDENT", [128, 128], BF16, kind="ExternalInput").ap()
    OUTT = nc.dram_tensor("OUTT", [D_MODEL, S], BF16, kind="ExternalOutput").ap()

    Exp = mybir.ActivationFunctionType.Exp
    Ident = mybir.ActivationFunctionType.Identity
    engs = (nc.sync, nc.scalar, nc.gpsimd)

    with tile.TileContext(nc) as tc:
        pp = tc.alloc_tile_pool(name="persist", bufs=1)
        qkp = tc.alloc_tile_pool(name="qkpool", bufs=1)
        vtp = tc.alloc_tile_pool(name="vtpool", bufs=1)
        xp = tc.alloc_tile_pool(name="xpool", bufs=1)
        wp = tc.alloc_tile_pool(name="wpool", bufs=1)
        mp = tc.alloc_tile_pool(name="mpool", bufs=3)
        ptp = tc.alloc_tile_pool(name="ptpool", bufs=12)
        # mask pool trimmed to fit xvh
        atp = tc.alloc_tile_pool(name="atpool", bufs=3)
        rvp = tc.alloc_tile_pool(name="rvpool", bufs=3)
        ps = tc.alloc_tile_pool(name="ps", bufs=1, space="PSUM")

        # ---- persistent small tensors ----
        bqk_t = pp.tile([128, 8], F32, name="bqk_t")
        bvon_t = pp.tile([1, 640], F32R, name="bvon_t")
        ident_t = pp.tile([128, 128], BF16, name="ident_t")
        bq_t, bk_t = bqk_t[:, 0:4], bqk_t[:, 4:8]
        bv_t, ones_t = bvon_t[:, 0:OG], bvon_t[:, OG:OG + 128]

        # ---- resident projection outputs ----
        qT = [qkp.tile([128, S], BF16, name=f"qT{j}") for j in range(4)]
        kT = [qkp.tile([128, S], BF16, name=f"kT{j}") for j in range(4)]
        onrm = [qkp.tile([128, S], BF16, name=f"onrm{c}") for c in range(4)]
        vt = [vtp.tile([128, HPG * 65], BF16, name=f"vt{j}") for j in range(16)]

        # ---- q/k x and w inputs (k first: K-proj gates the first scores) ----
        dengs = (nc.sync, nc.scalar, nc.gpsimd)
        xq_r = [xp.tile([128, IC * 512], BF16, name=f"xqr{r}")
                for r in range(4)]
        xk_r = [xp.tile([128, IC * 512], BF16, name=f"xkr{r}")
                for r in range(4)]

        def xk_ap(i, r):
            return xk_r[r][:, i * 512:(i + 1) * 512]

        def xq_ap(i, r):
            return xq_r[r][:, i * 512:(i + 1) * 512]

        wk_all = wp.tile([128, IC * OG], BF16, name="wk_all")
        wq_all = wp.tile([128, IC * OG], BF16, name="wq_all")
        wk = [wk_all[:, i * OG:(i + 1) * OG] for i in range(IC)]
        wq = [wq_all[:, i * OG:(i + 1) * OG] for i in range(IC)]
        # critical-path DMA order: wk, xk r0.., then the rest
        nc.sync.dma_start(out=wk_all[:], in_=WKT)
        for r in range(4):
            dengs[(1 + r) % 3].dma_start(out=xk_r[r][:], in_=XK3[r])
        nc.sync.dma_start(out=bqk_t[:], in_=BQK)
        nc.sync.dma_start(out=bvon_t[:], in_=BVON)
        nc.sync.dma_start(out=ident_t[:], in_=IDENT)

        # ---- mask tiles (keyed by tcc, reloaded each hp pass) ----
        mtiles = {}

        def load_mask(tcc):
            mh = mp.tile([128, S], BF16, tag="mask", name="mask")
            dengs[tcc % 2].dma_start(out=mh[:], in_=M4[tcc])
            mtiles[tcc] = mh

        # ---- projection building blocks ----
        pj_ctr = [0]

        def pj_psum():
            t_ = ps.tile([128, 512], F32, tag=f"pj{pj_ctr[0] % 2}",
                         name=f"pj{pj_ctr[0] % 2}")
            pj_ctr[0] += 1
            return t_

        def proj_block(dst, xs, ws, bias, j, r):
            """One (och j, t-round r) block of Q/K projection."""
            pj = pj_psum()
            for i in range(IC):
                nc.tensor.matmul(pj, ws(i, j), xs(i, r),
                                 start=(i == 0), stop=(i == IC - 1))
            nc.vector.tensor_scalar_add(dst[j][:, r * 512:(r + 1) * 512],
                                        pj, bias[:, j:j + 1])

        # ---------------- prologue ----------------
        load_mask(0)
        load_mask(1)
        for r in range(1, 4):
            dengs[r % 3].dma_start(out=xq_r[r][:], in_=XQ3[r])
        # xv loaded in two half-pools (released sequentially); wv in wp
        wv_all = wp.tile([128, IC * OG], BF16, name="wv_all")
        wv = [wv_all[:, i * OG:(i + 1) * OG] for i in range(IC)]
        nc.sync.dma_start(out=wv_all[:], in_=WVT)
        xvh = [None]

        def load_xv_half(hf):
            pool = tc.alloc_tile_pool(name=f"xvh{hf}", bufs=1)
            tls = [pool.tile([128, IC * 256], BF16, name=f"xv{hf}_{g}")
                   for g in range(4)]
            for g in range(4):
                dengs[(hf + g) % 3].dma_start(out=tls[g][:],
                                              in_=XV4[hf * 4 + g])
            xvh[0] = (pool, tls)

        load_xv_half(0)
        for tch in range(16):
            ocol = vt[tch][:].rearrange("p (h e) -> p h e", h=HPG)[:, :, 64:65]
            nc.vector.memset(ocol, 1.0)

        def vproj_block(tch):
            if tch == 8:
                xvh[0][0].release()
                load_xv_half(1)
            xvt = xvh[0][1][(tch % 8) // 2]
            c = (tch % 2) * 128
            pj = pj_psum()
            for i in range(IC):
                nc.tensor.matmul(pj, xvt[:, i * 256 + c:i * 256 + c + 128],
                                 wv[i], start=(i == 0), stop=False)
            nc.tensor.matmul(pj, ones_t, bv_t,
                             start=False, stop=True)
            nc.vector.tensor_copy(
                vt[tch][:].rearrange("p (h e) -> p h e", h=HPG)[:, :, 0:64],
                pj[:].rearrange("p (h d) -> p h d", h=HPG))

        proj_block(kT, xk_ap, wk_ap, bk_t, 0, 0)
        proj_block(qT, xq_ap, wq_ap, bq_t, 0, 0)
        proj_block(kT, xk_ap, wk_ap, bk_t, 0, 1)


        # ---------------- attention loop ----------------
        av = ps.tile([128, 1024], F32, tag="av", name="av")

        def score_group(h, tcc, g):
            ht, ho = h // 2, (h % 2) * 64
            mh = mtiles[tcc]
            s_ps = ps.tile([128, 1024], F32, tag=f"s{g}", name=f"s{g}")
            for sc8 in range(8):
                sc = g * 8 + sc8
                nc.tensor.matmul(
                    s_ps[:, sc8 * 128:(sc8 + 1) * 128],
                    kT[ht][ho:ho + 64, sc * 128:(sc + 1) * 128],
                    qT[ht][ho:ho + 64, tcc * 128:(tcc + 1) * 128],
                    start=True, stop=True)
            pt = ptp.tile([128, 1024], BF16, tag="pt", name="pt")
            nc.scalar.activation(pt[:], s_ps, Exp, scale=0.125)
            nc.vector.tensor_mul(pt[:], pt[:],
                                 mh[:, g * 1024:(g + 1) * 1024])
            return pt

        def unit_scores(h, tcc):
            return [score_group(h, tcc, 0), score_group(h, tcc, 1)]

        def unit_attnv(u):
            h, pts, uidx = u["h"], u["pts"], u["u"]
            c0 = (uidx % 8) * 128
            for g in range(2):
                for sc8 in range(8):
                    sc = g * 8 + sc8
                    nc.tensor.matmul(
                        av[:, c0:c0 + 65],
                        pts[g][:, sc8 * 128:(sc8 + 1) * 128],
                        vt[sc][:, h * 65:h * 65 + 65],
                        start=(sc == 0), stop=(sc == 15),
                        skip_group_check=True)
            rv = rvp.tile([128, 1], F32, tag="rv", name="rv")
            nc.vector.reciprocal(rv[:], av[:, c0 + 64:c0 + 65])
            at = atp.tile([128, 64], BF16, tag="at", name="at")
            nc.vector.tensor_scalar_mul(at[:], av[:, c0:c0 + 64], rv[:])
            u["at"] = at

        def unit_transp(u):
            # one stage later than unit_attnv: hides the DVE normalize chain
            h, tcc, at = u["h"], u["tcc"], u["at"]
            tp = pj_psum()
            tpb = tp[:].bitcast(BF16)
            pbase = (h % 2) * 64
            dst = tpb[pbase:pbase + 64, 0:128]
            nc.tensor.transpose(dst, at[:], ident_t[:])
            nc.vector.tensor_copy(
                onrm[h // 2][(h % 2) * 64:(h % 2) * 64 + 64,
                             tcc * 128:(tcc + 1) * 128], dst)
            if h == 7 and tcc % 4 == 3:
                for och in range(8):
                    steal.append(("outproj", tcc // 4, och))

        def outproj_block(w, och):
            op = pj_psum()
            for cch in range(4):
                nc.tensor.matmul(op, wo[cch][:, och * 128:(och + 1) * 128],
                                 onrm[cch][:, w * 512:(w + 1) * 512],
                                 start=(cch == 0), stop=(cch == 3))
            stg = sgp.tile([128, 512], BF16, tag="stg", name="stg")
            nc.vector.tensor_copy(stg[:], op[:])
            engs[och % 2].dma_start(
                out=OUTT.rearrange("(j p) m -> p j m", p=128)
                [:, och, w * 512:(w + 1) * 512], in_=stg[:])

        steal = [("vproj", tch) for tch in range(16)]
        steal += [("proj", qT, xq_ap, wq_ap, bq_t, 0, r) for r in (1, 2, 3)]
        for j in (1, 2, 3):
            for r in range(4):
                steal.append(("proj", kT, xk_ap, wk_ap, bk_t, j, r))
            for r in range(4):
                steal.append(("proj", qT, xq_ap, wq_ap, bq_t, j, r))

        nvproj = [16]

        def do_steal(it):
            if it[0] == "vproj":
                vproj_block(it[1])
                nvproj[0] -= 1
                if nvproj[0] == 0:
                    xvh[0][0].release()
            elif it[0] == "proj":
                _, dst, xs, ws, bias, j, r = it
                proj_block(dst, xs, ws, bias, j, r)
            else:
                outproj_block(it[1], it[2])

        pools_late = []
        wo = None
        sgp = None
        pend = []
        gpend = []
        tpend = []
        units = [(hp, tcc, h)
                 for hp in range(4)
                 for tcc in range(NTC)
                 for h in (2 * hp, 2 * hp + 1)]

        def process(u):
            """attnV/transp/steal work, one unit behind the scores issue."""
            if u < 8:
                nsteal = 3
            elif steal and steal[0][0] == "outproj":
                nsteal = 2 if len(steal) > 8 else 1
            else:
                nsteal = 1 if u % 4 == 3 else 0
            for _ in range(nsteal):
                if not steal:
                    break
                do_steal(steal.pop(0))
            pipe = 7 if nvproj[0] > 0 else 3
            while len(pend) > pipe:
                un = pend.pop(0)
                unit_attnv(un)
                tpend.append(un)
            while len(tpend) > 1:
                unit_transp(tpend.pop(0))

        # first unit: its g0 scores need only kT r0/r1, so the first exp
        # starts before K0 r2/r3's x transfers arrive
        pts0 = [score_group(0, 0, 0)]
        proj_block(kT, xk_ap, wk_ap, bk_t, 0, 2)
        proj_block(kT, xk_ap, wk_ap, bk_t, 0, 3)
        pts0.append(score_group(0, 0, 1))
        pend.append({"h": 0, "tcc": 0, "pts": pts0, "u": 0})

        for i, (hp, tcc, h) in enumerate(units):
            if (hp, tcc, h) == (3, 0, 6):
                # out-proj inputs: alloc late so they reuse released x space
                wop = tc.alloc_tile_pool(name="wopool", bufs=1)
                sgp = tc.alloc_tile_pool(name="stgpool", bufs=8)
                pools_late.extend([wop, sgp])
                wo = [wop.tile([128, D_MODEL], BF16, name=f"wo{j}")
                      for j in range(4)]
                for j in range(4):
                    nc.gpsimd.dma_start(out=wo[j][:],
                                        in_=WOT[j * 128:(j + 1) * 128, :])
            if h == 2 * hp:
                nxt = hp * NTC + tcc + 2
                if nxt < 4 * NTC:
                    load_mask(nxt % NTC)
            if i == 0:
                continue  # unit 0's scores were issued in the prologue
            # scores issue runs one unit AHEAD of processing: the next
            # unit's scores precede attnv/steal blocks in PE order, so the
            # exp ping-pong never waits on them
            pts = unit_scores(h, tcc)
            pend.append({"h": h, "tcc": tcc, "pts": pts, "u": i})
            if i >= 1:
                process(i - 1)
        process(len(units) - 1)
        while pend:
            un = pend.pop(0)
            unit_attnv(un)
            tpend.append(un)
        while tpend:
            unit_transp(tpend.pop(0))
        while steal:
            do_steal(steal.pop(0))

        # release pools in reverse alloc (stack) order
        for pool in list(reversed(pools_late)) + [ps, rvp, atp, ptp, mp,
                                                  wp, xp, vtp, qkp, pp]:
            pool.release()

    nc.compile()
    return nc


_NC_CACHE = {}


def _get_module():
    if "nc" not in _NC_CACHE:
        _NC_CACHE["nc"] = build_module()
    return _NC_CACHE["nc"]


def prepare_in_maps(q, k, v, mask, Wq, bq, Wk, bk, Wv, bv, Wo, bo):
    q = np.asarray(q, dtype=np.float32)
    k = np.asarray(k, dtype=np.float32)
    v = np.asarray(v, dtype=np.float32)
    mask = np.asarray(mask)
    Wq, Wk, Wv, Wo = (np.asarray(w, dtype=np.float32) for w in (Wq, Wk, Wv, Wo))
    bq, bk, bv, bo = (np.asarray(b_, dtype=np.float32) for b_ in (bq, bk, bv, bo))

    bf = ml_dtypes.bfloat16
    m = (mask[0, 0] != 0).astype(bf)
    m4 = np.ascontiguousarray(
        m.reshape(NTC, 128, NTC, 128).transpose(0, 3, 2, 1))
    ones = round_f32r(np.ones((1, 128), np.float32))
    ident = np.eye(128, dtype=bf)

    def pack_qk(x):
        # [S, D] -> x^T [D, S] -> [4, 128, IC*512] by r-block
        xt = x.T.astype(bf)
        return np.ascontiguousarray(
            xt.reshape(IC, 128, 4, 512).transpose(2, 1, 0, 3)
            .reshape(4, 128, IC * 512))

    def pack_v(x):
        xt = x.T.astype(bf)
        return np.ascontiguousarray(
            xt.reshape(IC, 128, 8, 256).transpose(2, 1, 0, 3)
            .reshape(8, 128, IC * 256))

    xT = {}
    for b_ in range(B):
        xT[("q", b_)] = pack_qk(q[b_])
        xT[("k", b_)] = pack_qk(k[b_])
        xT[("v", b_)] = pack_v(v[b_])
    def wtile(wmat, hg):
        # W^T slice [D_MODEL, OG] -> packed [128, IC*OG] (chunk i at cols i*OG)
        og = hg * OG
        wt = wmat[og:og + OG, :].T  # [D_MODEL, OG]
        return np.ascontiguousarray(
            wt.reshape(IC, 128, OG).transpose(1, 0, 2).reshape(128, IC * OG)
        ).astype(bf)

    def wtile4(wmat, hg):
        # by-och slab: [4, 128, IC*128]; slab j cols = och j*128..
        og = hg * OG
        wt = wmat[og:og + OG, :].T.astype(bf)  # [D_MODEL, OG]
        return np.ascontiguousarray(
            wt.reshape(IC, 128, 4, 128).transpose(2, 1, 0, 3)
            .reshape(4, 128, IC * 128))

    wslice = {}
    for hg in range(2):
        og = hg * OG
        wslice[("q", hg)] = wtile4(Wq, hg)
        wslice[("k", hg)] = wtile4(Wk, hg)
        wslice[("v", hg)] = wtile(Wv, hg)
        wslice[("o", hg)] = np.ascontiguousarray(Wo[:, og:og + OG].T).astype(bf)

    in_maps = []
    for c in range(NCORES):
        b_, hg = c // 2, c % 2
        og = hg * OG
        in_maps.append({
            "XQ3": xT[("q", b_)], "XK3": xT[("k", b_)], "XV4": xT[("v", b_)],
            "WQT": wslice[("q", hg)], "WKT": wslice[("k", hg)],
            "WVT": wslice[("v", hg)], "WOT": wslice[("o", hg)],
            "M4": m4,
            "BQK": np.concatenate([
                bq[og:og + OG].reshape(4, 128).T,
                bk[og:og + OG].reshape(4, 128).T], axis=1).astype(np.float32),
            "BVON": np.concatenate([
                round_f32r(bv[og:og + OG].reshape(1, OG)), ones], axis=1),
            "IDENT": ident,
        })

    return in_maps


def kernel(q, k, v, mask, Wq, bq, Wk, bk, Wv, bv, Wo, bo, **_ignored):
    bo = np.asarray(bo, dtype=np.float32)
    in_maps = prepare_in_maps(q, k, v, mask, Wq, bq, Wk, bk, Wv, bv, Wo, bo)
    nc = _get_module()
    res = run_bass_kernel_spmd(nc, in_maps, list(range(NCORES)))

    out = np.empty((B, S, D_MODEL), np.float32)
    for b_ in range(B):
        acc = (res.results[2 * b_]["OUTT"].astype(np.float32)
               + res.results[2 * b_ + 1]["OUTT"].astype(np.float32))
        out[b_] = acc.T + bo
    return out
